# revision 14
# baseline (speedup 1.0000x reference)
"""Trainium2 Bass kernel for nn_ODEModel (GNN message passing ODE, RK4).

Self-contained: hardcodes shapes from the problem spec; reads runtime values
(ts step, edge indices) from the actual input arrays at call time and bakes
them into the generated program.

Sharding: data-parallel over the 1024 independent systems -> 128 systems per
core across 8 NeuronCores. All MLP weights replicated. No cross-core comms.

Per-core layout (all activations "transposed", features on partitions):
  z state     zT [8, 1024]   col = obj*128 + sys        (obj-major)
  edge rows   [*, 7168]      col = edge*128 + sys       (edge-major)
  zpair [17, 8192]: rows 0:8 = z[o1], rows 8:16 = z[o2], row 16 = ones,
     col = (o1*8+o2)*128 + sys. The interaction-MLP layer-0 for edge e is ONE
     matmul vs zpair block p=rec[e]*8+snd[e] with lhsT = [A;B;b0] (17 x 512):
     A = [gW0_p; gW0_vrecv], B = [-gW0_p; gW0_vsend]. Consecutive edges with
     consecutive p indices are coalesced into single wider matmuls ("runs").
  Aggregation over the 7 senders per receiver is folded into the layer-2
  matmuls: 7 accumulating matmuls with strided rhs column access patterns.
Softplus = Ln(Exp(x) + 1) on the scalar engine (this toolchain has no native
softplus table); both funcs share one ACT table set.
Matmuls run in float32r (fp32 rounded to 11-bit mantissa, full PE rate).
"""
import numpy as np

import concourse.bass as bass
import concourse.bacc as bacc
import concourse.mybir as mybir
from concourse.tile import TileContext
from concourse.bass_utils import run_bass_kernel_spmd

F32 = mybir.dt.float32
F32R = mybir.dt.float32r
AF = mybir.ActivationFunctionType


def _pin_act_table_set():
    """Force the table-load pass to keep Exp and Ln in ONE act-func set
    (natural_log_exp_and_others). The rust pass picks the first set
    containing each function, which thrashes ~1.3us table reloads between
    every Exp and Ln otherwise. Dict order (= act_func_set_id) preserved."""
    import concourse.bacc as _bacc
    import concourse.hw_specs as _hws
    orig = _hws.get_activation_tables

    def patched(module_arch):
        full = dict(orig(module_arch))
        keep = "natural_log_exp_and_others"
        if keep in full and {AF.Exp, AF.Ln} <= full[keep]:
            out = {}
            for name, fns in full.items():
                if name != keep:
                    fns = fns - {AF.Exp, AF.Ln}
                out[name] = fns
            return out
        return full

    _bacc.get_activation_tables = patched


_pin_act_table_set()

B = 8           # objects per system
NF = 8          # state features (2n)
S = 128         # systems per core
NC = 8          # cores
E = 56          # edges per system
HI = 512        # interaction MLP hidden
HF = 256        # self MLP hidden
COLS = B * S            # 1024 object columns per core
ECOLS = E * S           # 7168 edge columns per core
NBLK_E = 4              # edge blocks per pipeline block (512 cols)
NBLKS = E // NBLK_E     # 14 pipeline blocks per stage
STEPS = 2               # RK4 steps (T-1)
N_BIAS_F32 = 3          # bias matmuls per block run in fp32 (PE ballast)


def round_fp32r(a):
    b = np.ascontiguousarray(a, dtype=np.float32).view(np.uint32)
    r = (b.astype(np.uint64) + 0x7FF + ((b >> 12) & 1)) & 0xFFFFF000
    return r.astype(np.uint32).view(np.float32)


def build_runs(rec_idx, snd_idx):
    """Maximal runs of consecutive edges whose zpair index p=rec*8+snd also
    increments by 1, chopped at 4-edge block boundaries. -> [(e0, p0, L)]"""
    p = [int(r) * 8 + int(s) for r, s in zip(rec_idx, snd_idx)]
    runs = []
    e = 0
    while e < E:
        e0, p0 = e, p[e]
        L = 1
        while e0 + L < E and p[e0 + L] == p0 + L and (e0 + L) % NBLK_E != 0:
            L += 1
        runs.append((e0, p0, L))
        e = e0 + L
    return runs


def build_program(h, runs):
    nc = bacc.Bacc("TRN2", target_bir_lowering=False, debug=False)

    zT0_d = nc.declare_dram_parameter("zT0", [NF, COLS], F32, isOutput=False)
    ab_d = nc.declare_dram_parameter("ab17", [17, HI], F32R, isOutput=False)
    w1g_d = nc.declare_dram_parameter("w1g", [HI, HI], F32R, isOutput=False)
    b1r_d = nc.declare_dram_parameter("b1row", [1, HI], F32R, isOutput=False)
    w2g_d = nc.declare_dram_parameter("w2g", [HI, NF], F32R, isOutput=False)
    w0f_d = nc.declare_dram_parameter("w0f", [NF, HF], F32R, isOutput=False)
    w1f_d = nc.declare_dram_parameter("w1f", [HF, HF], F32R, isOutput=False)
    w2f_d = nc.declare_dram_parameter("w2f", [HF, NF], F32R, isOutput=False)
    b0f_d = nc.declare_dram_parameter("b0f", [128, 2], F32, isOutput=False)
    b1f_d = nc.declare_dram_parameter("b1f", [128, 2], F32, isOutput=False)
    bk_d = nc.declare_dram_parameter("biask", [NF, 4], F32, isOutput=False)
    ones_d = nc.declare_dram_parameter("ones8k", [1, B * B * S], F32R, isOutput=False)
    y_d = nc.declare_dram_parameter("y", [STEPS, NF, COLS], F32, isOutput=True)

    with TileContext(nc) as tc:
        with tc.tile_pool(name="const", bufs=1) as cp, \
             tc.tile_pool(name="state", bufs=1) as sp, \
             tc.tile_pool(name="h2p", bufs=1) as h2p, \
             tc.tile_pool(name="h1p", bufs=2) as h1p, \
             tc.tile_pool(name="tmpp", bufs=3) as tp, \
             tc.tile_pool(name="smallp", bufs=2) as smp, \
             tc.tile_pool(name="mm0p", bufs=1, space="PSUM") as mm0p, \
             tc.tile_pool(name="mm2p", bufs=1, space="PSUM") as mm2p, \
             tc.tile_pool(name="aggp", bufs=2, space="PSUM") as aggp:

            # ---- persistent constants ----
            w_ab = cp.tile([17, HI], F32R, tag="w_ab")
            w1g = cp.tile([128, 4 * HI], F32R, tag="w1g")      # [:, kc*512+foc2*128]
            b1row = cp.tile([1, HI], F32R, tag="b1row")
            w2g = cp.tile([128, 4 * NF], F32R, tag="w2g")      # [:, kc*8]
            w0f = cp.tile([NF, HF], F32R, tag="w0f")
            w1f = cp.tile([128, 2 * HF], F32R, tag="w1f")      # [:, kc*256+foc2*128]
            w2f = cp.tile([128, 2 * NF], F32R, tag="w2f")      # [:, kc*8]
            b0f = cp.tile([128, 2], F32, tag="b0f")
            b1f = cp.tile([128, 2], F32, tag="b1f")
            bk = cp.tile([NF, 4], F32, tag="bk")
            ones = cp.tile([1, HI], F32R, tag="ones")

            nc.sync.dma_start(out=w_ab[:], in_=ab_d[:])
            for kc in range(4):
                nc.sync.dma_start(out=w1g[:, kc * HI:(kc + 1) * HI],
                                  in_=w1g_d[kc * 128:(kc + 1) * 128, :])
                nc.sync.dma_start(out=w2g[:, kc * NF:(kc + 1) * NF],
                                  in_=w2g_d[kc * 128:(kc + 1) * 128, :])
            nc.sync.dma_start(out=b1row[:], in_=b1r_d[:])
            nc.sync.dma_start(out=w0f[:], in_=w0f_d[:])
            for kc in range(2):
                nc.sync.dma_start(out=w1f[:, kc * HF:(kc + 1) * HF],
                                  in_=w1f_d[kc * 128:(kc + 1) * 128, :])
                nc.sync.dma_start(out=w2f[:, kc * NF:(kc + 1) * NF],
                                  in_=w2f_d[kc * 128:(kc + 1) * 128, :])
            nc.sync.dma_start(out=b0f[:], in_=b0f_d[:])
            nc.sync.dma_start(out=b1f[:], in_=b1f_d[:])
            nc.sync.dma_start(out=bk[:], in_=bk_d[:])
            nc.sync.dma_start(out=ones[:], in_=ones_d[0:1, 0:HI])

            # ---- persistent state ----
            zbase = sp.tile([NF, COLS], F32, tag="zbase")
            zround = sp.tile([NF, COLS], F32R, tag="zround")
            zcur = sp.tile([NF, COLS], F32R, tag="zcur")
            kacc = sp.tile([NF, COLS], F32, tag="kacc")
            zpair = sp.tile([17, B * B * S], F32R, tag="zpair")
            h2half = sp.tile([128, 4 * 28 * S], F32R, tag="h2half")
            h1f = sp.tile([128, 2 * COLS], F32R, tag="h1f")
            h2f = sp.tile([128, 2 * COLS], F32R, tag="h2f")

            nc.sync.dma_start(out=zbase[:], in_=zT0_d[:])
            nc.sync.dma_start(out=zpair[16:17, :], in_=ones_d[:])
            nc.scalar.activation(zround[:], zbase[:], AF.Copy)

            h2r = h2half[:].rearrange("p (k r j s) -> p k r j s",
                                      k=4, r=4, j=7, s=S)
            h2n = h2half[:].rearrange("p (k n c) -> p k n c",
                                      k=4, n=7, c=NBLK_E * S)

            for step in range(STEPS):
                for stage in range(4):
                    zin = zround if stage == 0 else zcur
                    zv = zin[:].rearrange("p (o s) -> p o s", s=S)

                    # ---- zpair build: 16 sbuf->sbuf DMAs ----
                    zp_hi = zpair[0:8, :].rearrange(
                        "p (a b s) -> p a b s", b=B, s=S)
                    zp_lo = zpair[8:16, :].rearrange(
                        "p (a b s) -> p a b s", b=B, s=S)
                    for o2 in range(B):
                        nc.sync.dma_start(out=zp_hi[:, :, o2, :], in_=zv)
                    for o1 in range(B):
                        nc.sync.dma_start(out=zp_lo[:, o1, :, :], in_=zv)

                    # ---- self MLP f on [8, 1024] ----
                    tmpf = tp.tile([128, 2 * COLS], F32, tag="tmp1")
                    for foc in range(2):
                        pf = mm2p.tile([128, COLS], F32, tag="mm2")
                        for nb in range(2):
                            nc.tensor.matmul(
                                pf[:, nb * HI:(nb + 1) * HI],
                                w0f[:, foc * 128:(foc + 1) * 128],
                                zin[:, nb * HI:(nb + 1) * HI],
                                start=True, stop=True)
                        nc.scalar.activation(
                            tmpf[:, foc * COLS:(foc + 1) * COLS], pf[:],
                            AF.Exp, bias=b0f[:, foc:foc + 1])
                    nc.scalar.activation(h1f[:], tmpf[:], AF.Ln, bias=1.0)

                    tmpf2 = tp.tile([128, 2 * COLS], F32, tag="tmp1")
                    for foc2 in range(2):
                        pf2 = mm2p.tile([128, COLS], F32, tag="mm2")
                        for nb in range(2):
                            for kc in range(2):
                                nc.tensor.matmul(
                                    pf2[:, nb * HI:(nb + 1) * HI],
                                    w1f[:, kc * HF + foc2 * 128:
                                        kc * HF + (foc2 + 1) * 128],
                                    h1f[:, kc * COLS + nb * HI:
                                        kc * COLS + (nb + 1) * HI],
                                    start=(kc == 0), stop=(kc == 1))
                        nc.scalar.activation(
                            tmpf2[:, foc2 * COLS:(foc2 + 1) * COLS], pf2[:],
                            AF.Exp, bias=b1f[:, foc2:foc2 + 1])
                    nc.scalar.activation(h2f[:], tmpf2[:], AF.Ln, bias=1.0)

                    # ---- interaction MLP pipeline + aggregation ----
                    paggs = []
                    for half in range(2):
                        for nb7 in range(7):
                            nblk = half * 7 + nb7
                            eb0 = nblk * NBLK_E
                            # l0g -> h1t [128, 4*512] (chunk kc = fo chunk)
                            h1t = h1p.tile([128, 4 * HI], F32R, tag="h1t")
                            tmp1 = tp.tile([128, 4 * HI], F32, tag="tmp1")
                            p0t = mm0p.tile([128, 4 * HI], F32, tag="mm0")
                            for foc in range(4):
                                for (e0, p0, L) in runs:
                                    if not (eb0 <= e0 < eb0 + NBLK_E):
                                        continue
                                    off = (e0 - eb0) * S
                                    nc.tensor.matmul(
                                        p0t[:, foc * HI + off:
                                            foc * HI + off + L * S],
                                        w_ab[:, foc * 128:(foc + 1) * 128],
                                        zpair[:, p0 * S:(p0 + L) * S],
                                        start=True, stop=True)
                            nc.scalar.activation(tmp1[:], p0t[:], AF.Exp)
                            nc.scalar.activation(h1t[:], tmp1[:], AF.Ln,
                                                 bias=1.0)

                            # l1g -> h2half columns for this nblk
                            tmp2 = tp.tile([128, 4 * HI], F32, tag="tmp1")
                            for fp_ in range(2):
                                p2t = mm2p.tile([128, 2 * HI], F32, tag="mm2")
                                for fi in range(2):
                                    foc2 = 2 * fp_ + fi
                                    for kc in range(4):
                                        nc.tensor.matmul(
                                            p2t[:, fi * HI:(fi + 1) * HI],
                                            w1g[:, kc * HI + foc2 * 128:
                                                kc * HI + (foc2 + 1) * 128],
                                            h1t[:, kc * HI:(kc + 1) * HI],
                                            start=(kc == 0), stop=False)
                                    if foc2 < N_BIAS_F32:
                                        # fp32 runs at 4 cyc/row: real work
                                        # used as PE ballast to keep the
                                        # tensor engine busy (HAM warm)
                                        nc.tensor.matmul(
                                            p2t[:, fi * HI:(fi + 1) * HI],
                                            b1row[:, foc2 * 128:
                                                  (foc2 + 1) * 128].bitcast(F32),
                                            ones[:].bitcast(F32),
                                            start=False, stop=True)
                                    else:
                                        nc.tensor.matmul(
                                            p2t[:, fi * HI:(fi + 1) * HI],
                                            b1row[:, foc2 * 128:(foc2 + 1) * 128],
                                            ones[:],
                                            start=False, stop=True)
                                nc.scalar.activation(
                                    tmp2[:, fp_ * 1024:(fp_ + 1) * 1024],
                                    p2t[:], AF.Exp)
                            nc.scalar.activation(
                                h2n[:, :, nb7, :],
                                tmp2[:].rearrange("p (k c) -> p k c",
                                                  c=NBLK_E * S),
                                AF.Ln, bias=1.0)

                        # l2f + l2agg for this half -> pagg [8, 512]
                        pagg = aggp.tile([NF, 4 * S], F32, tag="agg")
                        for kc in range(2):
                            nc.tensor.matmul(
                                pagg[:],
                                w2f[:, kc * NF:(kc + 1) * NF],
                                h2f[:, kc * COLS + half * 512:
                                    kc * COLS + (half + 1) * 512],
                                start=(kc == 0), stop=False)
                        for j in range(7):
                            for kc in range(4):
                                nc.tensor.matmul(
                                    pagg[:],
                                    w2g[:, kc * NF:(kc + 1) * NF],
                                    h2r[:, kc, :, j, :],
                                    start=False,
                                    stop=(j == 6 and kc == 3))
                        paggs.append(pagg)

                    # ---- RK4 stage tail ----
                    # kacc += w*k ; zcur = zbase + c*k
                    wcol = 0 if stage in (0, 3) else 1
                    wval = 1.0 if stage in (0, 3) else 2.0
                    if stage == 0:
                        for half in range(2):
                            nc.scalar.activation(
                                kacc[:, half * 512:(half + 1) * 512],
                                paggs[half][:], AF.Identity,
                                bias=bk[:, wcol:wcol + 1], scale=wval)
                    else:
                        tk = smp.tile([NF, COLS], F32, tag="tkz")
                        for half in range(2):
                            nc.scalar.activation(
                                tk[:, half * 512:(half + 1) * 512],
                                paggs[half][:], AF.Identity,
                                bias=bk[:, wcol:wcol + 1], scale=wval)
                        nc.vector.tensor_add(out=kacc[:], in0=kacc[:],
                                             in1=tk[:])
                    if stage < 3:
                        ccol = 2 if stage < 2 else 3
                        cval = h / 2 if stage < 2 else h
                        tz = smp.tile([NF, COLS], F32, tag="tkz")
                        for half in range(2):
                            nc.scalar.activation(
                                tz[:, half * 512:(half + 1) * 512],
                                paggs[half][:], AF.Identity,
                                bias=bk[:, ccol:ccol + 1], scale=cval)
                        nc.vector.tensor_add(out=zcur[:], in0=zbase[:],
                                             in1=tz[:])

                # ---- RK4 step tail: zbase += (h/6)*kacc ----
                tz = smp.tile([NF, COLS], F32, tag="tkz")
                nc.scalar.activation(tz[:], kacc[:], AF.Copy, scale=h / 6.0)
                nc.vector.tensor_add(out=zbase[:], in0=zbase[:], in1=tz[:])
                nc.sync.dma_start(out=y_d[step], in_=zbase[:])
                if step + 1 < STEPS:
                    nc.scalar.activation(zround[:], zbase[:], AF.Copy)

    nc.compile()
    return nc


def prepare_weights(inp, h):
    gW0 = np.asarray(inp['g_W0'], np.float32)          # [12, 512]
    ab17 = np.zeros((17, HI), np.float32)
    ab17[0:4] = gW0[0:4]
    ab17[4:8] = gW0[4:8]
    ab17[8:12] = -gW0[0:4]
    ab17[12:16] = gW0[8:12]
    ab17[16] = np.asarray(inp['g_b0'], np.float32)
    b2eff = (np.asarray(inp['f_b2'], np.float32)
             + 7.0 * np.asarray(inp['g_b2'], np.float32))
    biask = np.stack([b2eff, 2.0 * b2eff, (h / 2.0) * b2eff, h * b2eff],
                     axis=1).astype(np.float32)        # [8, 4]
    shared = {
        'ab17': round_fp32r(ab17),
        'w1g': round_fp32r(inp['g_W1']),
        'b1row': round_fp32r(np.asarray(inp['g_b1'],
                                        np.float32).reshape(1, HI)),
        'w2g': round_fp32r(inp['g_W2']),
        'w0f': round_fp32r(inp['f_W0']),
        'w1f': round_fp32r(inp['f_W1']),
        'w2f': round_fp32r(inp['f_W2']),
        'b0f': np.ascontiguousarray(
            np.asarray(inp['f_b0'], np.float32).reshape(2, 128).T),
        'b1f': np.ascontiguousarray(
            np.asarray(inp['f_b1'], np.float32).reshape(2, 128).T),
        'biask': biask,
        'ones8k': np.ones((1, B * B * S), np.float32),
    }
    return shared


def kernel(**inputs):
    inp = {k: np.asarray(v) for k, v in inputs.items()}
    zd0 = inp['zd_0'].astype(np.float32)               # [8192, 8]
    ts = np.asarray(inp['ts'], np.float32)
    h = float(ts[1] - ts[0])
    runs = build_runs(inp['rec_idx'], inp['send_idx'])

    nc = build_program(h, runs)
    shared = prepare_weights(inp, h)

    in_maps = []
    for c in range(NC):
        shard = zd0[c * COLS:(c + 1) * COLS]           # [1024, 8]
        zT0 = np.ascontiguousarray(
            shard.reshape(S, B, NF).transpose(2, 1, 0).reshape(NF, COLS))
        in_maps.append({'zT0': zT0, **shared})

    res = run_bass_kernel_spmd(nc, in_maps, core_ids=list(range(NC)))
    global LAST_RESULTS
    LAST_RESULTS = res

    NB = zd0.shape[0]
    out = np.empty((NB, STEPS + 1, NF), np.float32)
    out[:, 0, :] = zd0
    for c in range(NC):
        y = res.results[c]['y']                        # [2, 8, 1024]
        y = y.reshape(STEPS, NF, B, S).transpose(3, 2, 0, 1)
        out[c * COLS:(c + 1) * COLS, 1:, :] = y.reshape(COLS, STEPS, NF)
    return out


# revision 15
# speedup vs baseline: 1.5660x; 1.5660x over previous
"""Trainium2 Bass kernel for nn_ODEModel (GNN message passing ODE, RK4).

Self-contained: hardcodes shapes from the problem spec; reads runtime values
(ts step, edge indices) from the actual input arrays at call time and bakes
them into the generated program.

Sharding: data-parallel over the 1024 independent systems -> 128 systems per
core across 8 NeuronCores. All MLP weights replicated. No cross-core comms.

Per-core layout (all activations "transposed", features on partitions):
  z state     zT [8, 1024]   col = obj*128 + sys        (obj-major)
  edge rows   [*, 7168]      col = edge*128 + sys       (edge-major)
  zpair [17, 8192]: rows 0:8 = z[o1], rows 8:16 = z[o2], row 16 = ones,
     col = (o1*8+o2)*128 + sys. The interaction-MLP layer-0 for edge e is ONE
     matmul vs zpair block p=rec[e]*8+snd[e] with lhsT = [A;B;b0] (17 x 512):
     A = [gW0_p; gW0_vrecv], B = [-gW0_p; gW0_vsend]. Consecutive edges with
     consecutive p indices are coalesced into single wider matmuls ("runs").
  Aggregation over the 7 senders per receiver is folded into the layer-2
  matmuls: 7 accumulating matmuls with strided rhs column access patterns.
Softplus = Ln(Exp(x) + 1) on the scalar engine (this toolchain has no native
softplus table); both funcs share one ACT table set.
Matmuls run in float32r (fp32 rounded to 11-bit mantissa, full PE rate).
"""
import numpy as np

import concourse.bass as bass
import concourse.bacc as bacc
import concourse.mybir as mybir
from concourse.tile import TileContext
from concourse.bass_utils import run_bass_kernel_spmd

F32 = mybir.dt.float32
F32R = mybir.dt.float32r
AF = mybir.ActivationFunctionType


def _pin_act_table_set():
    """Force the table-load pass to keep Exp and Ln in ONE act-func set
    (natural_log_exp_and_others). The rust pass picks the first set
    containing each function, which thrashes ~1.3us table reloads between
    every Exp and Ln otherwise. Dict order (= act_func_set_id) preserved."""
    import concourse.bacc as _bacc
    import concourse.hw_specs as _hws
    orig = _hws.get_activation_tables

    def patched(module_arch):
        full = dict(orig(module_arch))
        keep = "natural_log_exp_and_others"
        if keep in full and {AF.Exp, AF.Ln} <= full[keep]:
            out = {}
            for name, fns in full.items():
                if name != keep:
                    fns = fns - {AF.Exp, AF.Ln}
                out[name] = fns
            return out
        return full

    _bacc.get_activation_tables = patched


_pin_act_table_set()

B = 8           # objects per system
NF = 8          # state features (2n)
S = 128         # systems per core
NC = 8          # cores
E = 56          # edges per system
HI = 512        # interaction MLP hidden
HF = 256        # self MLP hidden
COLS = B * S            # 1024 object columns per core
ECOLS = E * S           # 7168 edge columns per core
NBLK_E = 4              # edge blocks per pipeline block (512 cols)
NBLKS = E // NBLK_E     # 14 pipeline blocks per stage
STEPS = 2               # RK4 steps (T-1)


def round_fp32r(a):
    b = np.ascontiguousarray(a, dtype=np.float32).view(np.uint32)
    r = (b.astype(np.uint64) + 0x7FF + ((b >> 12) & 1)) & 0xFFFFF000
    return r.astype(np.uint32).view(np.float32)


def build_runs(rec_idx, snd_idx):
    """Maximal runs of consecutive edges whose zpair index p=rec*8+snd also
    increments by 1, chopped at 4-edge block boundaries. -> [(e0, p0, L)]"""
    p = [int(r) * 8 + int(s) for r, s in zip(rec_idx, snd_idx)]
    runs = []
    e = 0
    while e < E:
        e0, p0 = e, p[e]
        L = 1
        while e0 + L < E and p[e0 + L] == p0 + L and (e0 + L) % NBLK_E != 0:
            L += 1
        runs.append((e0, p0, L))
        e = e0 + L
    return runs


def build_program(h, runs):
    nc = bacc.Bacc("TRN2", target_bir_lowering=False, debug=False)

    zT0_d = nc.declare_dram_parameter("zT0", [NF, COLS], F32, isOutput=False)
    ab_d = nc.declare_dram_parameter("ab17", [17, HI], F32R, isOutput=False)
    w1g_d = nc.declare_dram_parameter("w1g", [HI, HI], F32R, isOutput=False)
    b1g_d = nc.declare_dram_parameter("b1g", [128, 4], F32, isOutput=False)
    w2g_d = nc.declare_dram_parameter("w2g", [HI, NF], F32R, isOutput=False)
    w0f_d = nc.declare_dram_parameter("w0f", [NF, HF], F32R, isOutput=False)
    w1f_d = nc.declare_dram_parameter("w1f", [HF, HF], F32R, isOutput=False)
    w2f_d = nc.declare_dram_parameter("w2f", [HF, NF], F32R, isOutput=False)
    b0f_d = nc.declare_dram_parameter("b0f", [128, 2], F32, isOutput=False)
    b1f_d = nc.declare_dram_parameter("b1f", [128, 2], F32, isOutput=False)
    bk_d = nc.declare_dram_parameter("biask", [NF, 4], F32, isOutput=False)
    ones_d = nc.declare_dram_parameter("ones8k", [1, B * B * S], F32R, isOutput=False)
    y_d = nc.declare_dram_parameter("y", [STEPS, NF, COLS], F32, isOutput=True)

    with TileContext(nc) as tc:
        with tc.tile_pool(name="const", bufs=1) as cp, \
             tc.tile_pool(name="state", bufs=1) as sp, \
             tc.tile_pool(name="h2p", bufs=1) as h2p, \
             tc.tile_pool(name="h1p", bufs=2) as h1p, \
             tc.tile_pool(name="tmpp", bufs=3) as tp, \
             tc.tile_pool(name="smallp", bufs=2) as smp, \
             tc.tile_pool(name="mm0p", bufs=1, space="PSUM") as mm0p, \
             tc.tile_pool(name="mm2p", bufs=2, space="PSUM") as mm2p, \
             tc.tile_pool(name="aggp", bufs=2, space="PSUM") as aggp:

            # ---- persistent constants ----
            w_ab = cp.tile([96 + 17, HI], F32R, tag="w_ab")
            w1g = cp.tile([128, 4 * HI], F32R, tag="w1g")      # [:, kc*512+foc2*128]
            b1g = cp.tile([128, 4], F32, tag="b1g")
            w2g = cp.tile([128, 4 * NF], F32R, tag="w2g")      # [:, kc*8]
            w0f = cp.tile([NF, HF], F32R, tag="w0f")
            w1f = cp.tile([128, 2 * HF], F32R, tag="w1f")      # [:, kc*256+foc2*128]
            w2f = cp.tile([128, 2 * NF], F32R, tag="w2f")      # [:, kc*8]
            b0f = cp.tile([128, 2], F32, tag="b0f")
            b1f = cp.tile([128, 2], F32, tag="b1f")
            bk = cp.tile([NF, 4], F32, tag="bk")

            for rg in range(4):
                nc.sync.dma_start(out=w_ab[32 * rg:32 * rg + 17, :],
                                  in_=ab_d[:])
            for kc in range(4):
                nc.sync.dma_start(out=w1g[:, kc * HI:(kc + 1) * HI],
                                  in_=w1g_d[kc * 128:(kc + 1) * 128, :])
                nc.sync.dma_start(out=w2g[:, kc * NF:(kc + 1) * NF],
                                  in_=w2g_d[kc * 128:(kc + 1) * 128, :])
            nc.sync.dma_start(out=b1g[:], in_=b1g_d[:])
            nc.sync.dma_start(out=w0f[:], in_=w0f_d[:])
            for kc in range(2):
                nc.sync.dma_start(out=w1f[:, kc * HF:(kc + 1) * HF],
                                  in_=w1f_d[kc * 128:(kc + 1) * 128, :])
                nc.sync.dma_start(out=w2f[:, kc * NF:(kc + 1) * NF],
                                  in_=w2f_d[kc * 128:(kc + 1) * 128, :])
            nc.sync.dma_start(out=b0f[:], in_=b0f_d[:])
            nc.sync.dma_start(out=b1f[:], in_=b1f_d[:])
            nc.sync.dma_start(out=bk[:], in_=bk_d[:])

            # ---- persistent state ----
            zbase = sp.tile([NF, COLS], F32, tag="zbase")
            zround = sp.tile([NF, COLS], F32R, tag="zround")
            zcur = sp.tile([NF, COLS], F32R, tag="zcur")
            kacc = sp.tile([NF, COLS], F32, tag="kacc")
            zpair = sp.tile([96 + 17, B * B * S], F32R, tag="zpair")
            h2half = sp.tile([128, 4 * 28 * S], F32R, tag="h2half")
            h1f = sp.tile([128, 2 * COLS], F32R, tag="h1f")
            h2f = sp.tile([128, 2 * COLS], F32R, tag="h2f")

            nc.sync.dma_start(out=zbase[:], in_=zT0_d[:])
            for rg in range(4):
                nc.sync.dma_start(out=zpair[32 * rg + 16:32 * rg + 17, :],
                                  in_=ones_d[:])
            nc.scalar.activation(zround[:], zbase[:], AF.Copy)

            h2r = h2half[:].rearrange("p (k r j s) -> p k r j s",
                                      k=4, r=4, j=7, s=S)
            h2n = h2half[:].rearrange("p (k n c) -> p k n c",
                                      k=4, n=7, c=NBLK_E * S)

            for step in range(STEPS):
                for stage in range(4):
                    zin = zround if stage == 0 else zcur
                    zv = zin[:].rearrange("p (o s) -> p o s", s=S)

                    # ---- zpair build: 16 sbuf->sbuf DMAs x 4 row groups ----
                    for rg in range(4):
                        zp_hi = zpair[32 * rg:32 * rg + 8, :].rearrange(
                            "p (a b s) -> p a b s", b=B, s=S)
                        zp_lo = zpair[32 * rg + 8:32 * rg + 16, :].rearrange(
                            "p (a b s) -> p a b s", b=B, s=S)
                        for o2 in range(B):
                            nc.sync.dma_start(out=zp_hi[:, :, o2, :], in_=zv)
                        for o1 in range(B):
                            nc.sync.dma_start(out=zp_lo[:, o1, :, :], in_=zv)

                    # ---- self MLP f on [8, 1024] ----
                    tmpf = tp.tile([128, 2 * COLS], F32, tag="tmp1")
                    pf = mm0p.tile([128, 4 * HI], F32, tag="mm0")
                    for foc in range(2):
                        for nb in range(2):
                            nc.tensor.matmul(
                                pf[:, foc * COLS + nb * HI:
                                   foc * COLS + (nb + 1) * HI],
                                w0f[:, foc * 128:(foc + 1) * 128],
                                zin[:, nb * HI:(nb + 1) * HI],
                                start=True, stop=True)
                    for foc in range(2):
                        nc.scalar.activation(
                            tmpf[:, foc * COLS:(foc + 1) * COLS],
                            pf[:, foc * COLS:(foc + 1) * COLS],
                            AF.Exp, bias=b0f[:, foc:foc + 1])
                    nc.scalar.activation(h1f[:], tmpf[:], AF.Ln, bias=1.0)

                    tmpf2 = tp.tile([128, 2 * COLS], F32, tag="tmp1")
                    pf2 = mm0p.tile([128, 4 * HI], F32, tag="mm0")
                    for foc2 in range(2):
                        for nb in range(2):
                            for kc in range(2):
                                nc.tensor.matmul(
                                    pf2[:, foc2 * COLS + nb * HI:
                                        foc2 * COLS + (nb + 1) * HI],
                                    w1f[:, kc * HF + foc2 * 128:
                                        kc * HF + (foc2 + 1) * 128],
                                    h1f[:, kc * COLS + nb * HI:
                                        kc * COLS + (nb + 1) * HI],
                                    start=(kc == 0), stop=(kc == 1))
                    for foc2 in range(2):
                        nc.scalar.activation(
                            tmpf2[:, foc2 * COLS:(foc2 + 1) * COLS],
                            pf2[:, foc2 * COLS:(foc2 + 1) * COLS],
                            AF.Exp, bias=b1f[:, foc2:foc2 + 1])
                    nc.scalar.activation(h2f[:], tmpf2[:], AF.Ln, bias=1.0)

                    # ---- interaction MLP pipeline + aggregation ----
                    paggs = []
                    for half in range(2):
                        for nb7 in range(7):
                            nblk = half * 7 + nb7
                            eb0 = nblk * NBLK_E
                            # l0g -> h1t [128, 4*512] (chunk kc = fo chunk)
                            h1t = h1p.tile([128, 4 * HI], F32R, tag="h1t")
                            tmp1 = tp.tile([128, 4 * HI], F32, tag="tmp1")
                            p0t = mm0p.tile([128, 4 * HI], F32, tag="mm0")
                            for foc in range(4):
                                rg = 32 * foc
                                for (e0, p0, L) in runs:
                                    if not (eb0 <= e0 < eb0 + NBLK_E):
                                        continue
                                    off = (e0 - eb0) * S
                                    nc.tensor.matmul(
                                        p0t[:, foc * HI + off:
                                            foc * HI + off + L * S],
                                        w_ab[rg:rg + 17,
                                             foc * 128:(foc + 1) * 128],
                                        zpair[rg:rg + 17,
                                              p0 * S:(p0 + L) * S],
                                        start=True, stop=True,
                                        tile_position=(rg, 0))
                            nc.scalar.activation(tmp1[:], p0t[:], AF.Exp)
                            nc.scalar.activation(h1t[:], tmp1[:], AF.Ln,
                                                 bias=1.0)

                            # l1g -> h2half columns for this nblk
                            tmp2 = tp.tile([128, 4 * HI], F32, tag="tmp1")
                            for foc2 in range(4):
                                p2t = mm2p.tile([128, HI], F32, tag="mm2")
                                for kc in range(4):
                                    nc.tensor.matmul(
                                        p2t[:],
                                        w1g[:, kc * HI + foc2 * 128:
                                            kc * HI + (foc2 + 1) * 128],
                                        h1t[:, kc * HI:(kc + 1) * HI],
                                        start=(kc == 0), stop=(kc == 3))
                                nc.scalar.activation(
                                    tmp2[:, foc2 * HI:(foc2 + 1) * HI],
                                    p2t[:], AF.Exp,
                                    bias=b1g[:, foc2:foc2 + 1])
                            nc.scalar.activation(
                                h2n[:, :, nb7, :],
                                tmp2[:].rearrange("p (k c) -> p k c",
                                                  c=NBLK_E * S),
                                AF.Ln, bias=1.0)

                        # l2f + l2agg for this half -> pagg [8, 512]
                        pagg = aggp.tile([NF, 4 * S], F32, tag="agg")
                        for kc in range(2):
                            nc.tensor.matmul(
                                pagg[:],
                                w2f[:, kc * NF:(kc + 1) * NF],
                                h2f[:, kc * COLS + half * 512:
                                    kc * COLS + (half + 1) * 512],
                                start=(kc == 0), stop=False)
                        for j in range(7):
                            for kc in range(4):
                                nc.tensor.matmul(
                                    pagg[:],
                                    w2g[:, kc * NF:(kc + 1) * NF],
                                    h2r[:, kc, :, j, :],
                                    start=False,
                                    stop=(j == 6 and kc == 3))
                        paggs.append(pagg)

                    # ---- RK4 stage tail ----
                    # kacc += w*k ; zcur = zbase + c*k
                    wcol = 0 if stage in (0, 3) else 1
                    wval = 1.0 if stage in (0, 3) else 2.0
                    if stage == 0:
                        for half in range(2):
                            nc.scalar.activation(
                                kacc[:, half * 512:(half + 1) * 512],
                                paggs[half][:], AF.Identity,
                                bias=bk[:, wcol:wcol + 1], scale=wval)
                    else:
                        tk = smp.tile([NF, COLS], F32, tag="tkz")
                        for half in range(2):
                            nc.scalar.activation(
                                tk[:, half * 512:(half + 1) * 512],
                                paggs[half][:], AF.Identity,
                                bias=bk[:, wcol:wcol + 1], scale=wval)
                        nc.vector.tensor_add(out=kacc[:], in0=kacc[:],
                                             in1=tk[:])
                    if stage < 3:
                        ccol = 2 if stage < 2 else 3
                        cval = h / 2 if stage < 2 else h
                        tz = smp.tile([NF, COLS], F32, tag="tkz")
                        for half in range(2):
                            nc.scalar.activation(
                                tz[:, half * 512:(half + 1) * 512],
                                paggs[half][:], AF.Identity,
                                bias=bk[:, ccol:ccol + 1], scale=cval)
                        nc.vector.tensor_add(out=zcur[:], in0=zbase[:],
                                             in1=tz[:])

                # ---- RK4 step tail: zbase += (h/6)*kacc ----
                tz = smp.tile([NF, COLS], F32, tag="tkz")
                nc.scalar.activation(tz[:], kacc[:], AF.Copy, scale=h / 6.0)
                nc.vector.tensor_add(out=zbase[:], in0=zbase[:], in1=tz[:])
                nc.sync.dma_start(out=y_d[step], in_=zbase[:])
                if step + 1 < STEPS:
                    nc.scalar.activation(zround[:], zbase[:], AF.Copy)

    nc.compile()
    return nc


def prepare_weights(inp, h):
    gW0 = np.asarray(inp['g_W0'], np.float32)          # [12, 512]
    ab17 = np.zeros((17, HI), np.float32)
    ab17[0:4] = gW0[0:4]
    ab17[4:8] = gW0[4:8]
    ab17[8:12] = -gW0[0:4]
    ab17[12:16] = gW0[8:12]
    ab17[16] = np.asarray(inp['g_b0'], np.float32)
    b2eff = (np.asarray(inp['f_b2'], np.float32)
             + 7.0 * np.asarray(inp['g_b2'], np.float32))
    biask = np.stack([b2eff, 2.0 * b2eff, (h / 2.0) * b2eff, h * b2eff],
                     axis=1).astype(np.float32)        # [8, 4]
    shared = {
        'ab17': round_fp32r(ab17),
        'w1g': round_fp32r(inp['g_W1']),
        'b1g': np.ascontiguousarray(
            np.asarray(inp['g_b1'], np.float32).reshape(4, 128).T),
        'w2g': round_fp32r(inp['g_W2']),
        'w0f': round_fp32r(inp['f_W0']),
        'w1f': round_fp32r(inp['f_W1']),
        'w2f': round_fp32r(inp['f_W2']),
        'b0f': np.ascontiguousarray(
            np.asarray(inp['f_b0'], np.float32).reshape(2, 128).T),
        'b1f': np.ascontiguousarray(
            np.asarray(inp['f_b1'], np.float32).reshape(2, 128).T),
        'biask': biask,
        'ones8k': np.ones((1, B * B * S), np.float32),
    }
    return shared


def kernel(**inputs):
    inp = {k: np.asarray(v) for k, v in inputs.items()}
    zd0 = inp['zd_0'].astype(np.float32)               # [8192, 8]
    ts = np.asarray(inp['ts'], np.float32)
    h = float(ts[1] - ts[0])
    runs = build_runs(inp['rec_idx'], inp['send_idx'])

    nc = build_program(h, runs)
    shared = prepare_weights(inp, h)

    in_maps = []
    for c in range(NC):
        shard = zd0[c * COLS:(c + 1) * COLS]           # [1024, 8]
        zT0 = np.ascontiguousarray(
            shard.reshape(S, B, NF).transpose(2, 1, 0).reshape(NF, COLS))
        in_maps.append({'zT0': zT0, **shared})

    res = run_bass_kernel_spmd(nc, in_maps, core_ids=list(range(NC)))
    global LAST_RESULTS
    LAST_RESULTS = res

    NB = zd0.shape[0]
    out = np.empty((NB, STEPS + 1, NF), np.float32)
    out[:, 0, :] = zd0
    for c in range(NC):
        y = res.results[c]['y']                        # [2, 8, 1024]
        y = y.reshape(STEPS, NF, B, S).transpose(3, 2, 0, 1)
        out[c * COLS:(c + 1) * COLS, 1:, :] = y.reshape(COLS, STEPS, NF)
    return out
